# revision 1
# baseline (speedup 1.0000x reference)
"""Causal self-attention (B=2, T=2048, E=1024, H=16, d_k=64) on 8 TRN2 cores.

Sharding: 2D (head-group x batch). Core c = (b, g) with b = c // 4,
g = c % 4 owns heads 4g..4g+3 (feature slice 256g:256g+256) of batch b.
Each core computes a partial output [2048, 1024] (its 4 heads'
contribution, post-Wo); the host sums the 4 partials per batch and
adds bo.

All matmul operands are bf16 (fp32 PSUM accumulation). Per-core
pipeline, fused over 512-token query chunks:
  B: project x -> qT/kT (features on partitions) and V (natural
     [token, feature] layout via per-token-group matmuls; bias folded
     in as a K=1 rank-1 matmul; ones column appended per head for the
     softmax denominator).
  C: per head pair (2 heads sharing the PE array via K=64 row tiles
     at partitions 0:64 / 64:128): scores -> exp (mask applied after
     exp as a 0/1 multiply; safe because logits are O(1) here) ->
     prob @ V accumulating [y; denom] in PSUM -> normalize by
     broadcasting 1/denom with a rank-1 matmul.
  D: output projection of the finished token chunk, partial written
     as bf16.
"""

import numpy as np

B = 2
T = 2048
E = 1024
F = 256          # per-core features (4 heads x 64)
DK = 64
NH_LOC = 4       # heads per core
N_CORES = 8
IC = 512         # query chunk (moving free dim)
JC = 128         # key chunk (stationary free dim)
N_IC = T // IC   # query chunks per batch
N_TC = T // JC   # 128-token chunks per batch
N_EC = E // 128  # contraction chunks
TT = B * T

_CACHE = {}


def _build_program(debug_taps=False):
    import concourse.mybir as mybir
    import concourse.tile as tile
    from concourse import bacc

    f32 = mybir.dt.float32
    bf16 = mybir.dt.bfloat16
    Act = mybir.ActivationFunctionType

    nc = bacc.Bacc("TRN2", target_bir_lowering=False, debug=False)

    x_ap = nc.dram_tensor("x", [E, T], bf16, kind="ExternalInput").ap()
    wqT = nc.dram_tensor("wqT", [E, F], bf16, kind="ExternalInput").ap()
    wkT = nc.dram_tensor("wkT", [E, F], bf16, kind="ExternalInput").ap()
    wvT = nc.dram_tensor("wvT", [E, F], bf16, kind="ExternalInput").ap()
    woT = nc.dram_tensor("woT", [F, E], bf16, kind="ExternalInput").ap()
    bq_ap = nc.dram_tensor("bq", [F], f32, kind="ExternalInput").ap()
    bk_ap = nc.dram_tensor("bk", [F], f32, kind="ExternalInput").ap()
    bv_ap = nc.dram_tensor("bv", [F], bf16, kind="ExternalInput").ap()
    masks_ap = nc.dram_tensor("masks", [JC, JC], bf16, kind="ExternalInput").ap()
    out_ap = nc.dram_tensor("partial", [T, E], bf16, kind="ExternalOutput").ap()
    if debug_taps:
        dbg_qt = nc.dram_tensor("dbg_qt", [128, 2, T], f32, kind="ExternalOutput").ap()
        dbg_kt = nc.dram_tensor("dbg_kt", [128, 2, T], f32, kind="ExternalOutput").ap()
        dbg_v1 = nc.dram_tensor("dbg_v1", [128, N_TC, 4 * 66], f32, kind="ExternalOutput").ap()
        dbg_yt = nc.dram_tensor("dbg_yt", [128, 2, T], f32, kind="ExternalOutput").ap()

    with tile.TileContext(nc) as tc:
        with (
            tc.tile_pool(name="const", bufs=1) as constp,
            tc.tile_pool(name="persist", bufs=1) as persist,
            tc.tile_pool(name="xt", bufs=2) as xtp,
            tc.tile_pool(name="pt", bufs=5) as ptp,
            tc.tile_pool(name="work", bufs=6) as work,
            tc.tile_pool(name="outs", bufs=4) as outsp,
            tc.tile_pool(name="ps_sc", bufs=2, space="PSUM") as ps_sc,
            tc.tile_pool(name="ps_y", bufs=2, space="PSUM") as ps_y,
            tc.tile_pool(name="ps_w", bufs=2, space="PSUM") as ps_w,
        ):
            # ---- tiles ----
            wq_sb = constp.tile([128, N_EC, F], bf16, tag="wq")
            wk_sb = constp.tile([128, N_EC, F], bf16, tag="wk")
            wv_sb = constp.tile([128, N_EC, F], bf16, tag="wv")
            wo_sb = constp.tile([128, 2, E], bf16, tag="wo")
            bq_sb = constp.tile([128, 2], f32, tag="bq")
            bk_sb = constp.tile([128, 2], f32, tag="bk")
            bv_sb = constp.tile([1, F], bf16, tag="bv")
            masks_sb = constp.tile([128, 1, JC], bf16, tag="masks")
            ones_f32 = constp.tile([128, 1], f32, tag="ones_f32")
            ones1 = constp.tile([1, 128], bf16, tag="ones1")
            ones64 = constp.tile([1, DK], bf16, tag="ones64")
            ones_row = ones_f32[:, 0:1].broadcast_to([128, IC])

            qt_sb = persist.tile([128, 2, T], bf16, tag="qt")   # [f, ft, t]
            kt_sb = persist.tile([128, 2, T], bf16, tag="kt")
            # V natural layout + ones col: head h data at cols h*66..h*66+63,
            # ones at h*66+64, pad at h*66+65
            v1_sb = persist.tile([128, N_TC, NH_LOC * 66], bf16, tag="v1")
            yt_sb = persist.tile([128, 2, T], bf16, tag="yt")

            xt_re = x_ap.rearrange("(a p) t -> p a t", p=128)
            xts = {}

            # DMAs in order of first use; x chunk 0 ahead of everything
            def dma_x(ic):
                xt = xtp.tile([128, N_EC, IC], bf16, tag="xt")
                nc.sync.dma_start(xt[:], xt_re[:, :, ic * IC : (ic + 1) * IC])
                xts[ic] = xt

            wq_re = wqT.rearrange("(a p) f -> p a f", p=128)
            nc.sync.dma_start(wq_sb[:, 0:4], wq_re[:, 0:4])
            nc.sync.dma_start(bq_sb[:], bq_ap.rearrange("(a p) -> p a", p=128))
            xt0 = xtp.tile([128, N_EC, IC], bf16, tag="xt")
            xts[0] = xt0
            nc.sync.dma_start(xt0[:, 0:4], xt_re[:, 0:4, 0:IC])
            nc.sync.dma_start(wq_sb[:, 4:8], wq_re[:, 4:8])
            nc.sync.dma_start(xt0[:, 4:8], xt_re[:, 4:8, 0:IC])
            nc.sync.dma_start(wk_sb[:], wkT.rearrange("(a p) f -> p a f", p=128))
            nc.sync.dma_start(bk_sb[:], bk_ap.rearrange("(a p) -> p a", p=128))
            nc.sync.dma_start(wv_sb[:], wvT.rearrange("(a p) f -> p a f", p=128))
            nc.sync.dma_start(bv_sb[:], bv_ap.rearrange("(p f) -> p f", p=1))
            nc.sync.dma_start(masks_sb[:], masks_ap.rearrange("p (o i) -> p o i", o=1))
            nc.sync.dma_start(wo_sb[:], woT.rearrange("(a p) f -> p a f", p=128))
            nc.vector.memset(ones_f32[:], 1.0)
            nc.vector.memset(ones1[:], 1.0)
            nc.vector.memset(ones64[:], 1.0)
            for h in range(NH_LOC):
                nc.vector.memset(v1_sb[:, :, h * 66 + 64], 1.0)
                nc.vector.memset(v1_sb[:, :, h * 66 + 65], 0.0)

            # ---- work units (closures) for pipelined emission; split into
            # ~1us sub-units sharing state so pacing stays smooth ----
            def b_qk_unit(ic, dst_sb, w_sb, b_sb, ft):
                st = {}

                def emit_a():
                    ps = ps_w.tile([128, IC], f32, tag="psb")
                    st["ps"] = ps
                    for ec in range(N_EC // 2):
                        nc.tensor.matmul(
                            ps[:],
                            w_sb[:, ec, ft * 128 : (ft + 1) * 128],
                            xts[ic][:, ec, :],
                            start=(ec == 0),
                            stop=False,
                        )

                def emit_b():
                    t0 = ic * IC
                    ps = st["ps"]
                    for ec in range(N_EC // 2, N_EC):
                        nc.tensor.matmul(
                            ps[:],
                            w_sb[:, ec, ft * 128 : (ft + 1) * 128],
                            xts[ic][:, ec, :],
                            start=False,
                            stop=(ec == N_EC - 1),
                        )
                    nc.vector.scalar_tensor_tensor(
                        dst_sb[:, ft, t0 : t0 + IC],
                        ps[:], b_sb[:, ft : ft + 1], ones_row,
                        op0=mybir.AluOpType.add, op1=mybir.AluOpType.mult,
                    )
                return [emit_a, emit_b]

            def b_v_unit(ic, g):
                def emit():
                    vps = ps_w.tile([128, F], f32, tag="psb")
                    for ec in range(N_EC):
                        nc.tensor.matmul(
                            vps[:],
                            xts[ic][:, ec, g * 128 : (g + 1) * 128],
                            wv_sb[:, ec, :],
                            start=(ec == 0),
                            stop=False,
                        )
                    nc.tensor.matmul(
                        vps[:], ones1[:], bv_sb[:], start=False, stop=True
                    )
                    tci = ic * (IC // 128) + g
                    nc.vector.tensor_copy(
                        v1_sb[:, tci, :]
                        .rearrange("p (h c) -> p h c", h=NH_LOC)[:, :, 0:DK],
                        vps.rearrange("p (h c) -> p h c", c=DK),
                    )
                return [emit]

            def b_units(ic):
                units = []
                for dst_sb, w_sb, b_sb in ((qt_sb, wq_sb, bq_sb), (kt_sb, wk_sb, bk_sb)):
                    for ft in range(2):
                        units += b_qk_unit(ic, dst_sb, w_sb, b_sb, ft)
                for g in range(IC // 128):
                    units += b_v_unit(ic, g)
                return units

            def d_unit(ic, g):
                st = {}
                tail = ic == N_IC - 1

                def emit_fc(fc):
                    tg = ic * IC + g * 128
                    f0 = fc * (E // 2)
                    if fc == 0:
                        st["ob"] = outsp.tile([128, E], bf16, tag="ob", name="ob")
                    ob = st["ob"]
                    ops = ps_w.tile([128, E // 2], f32, tag="psb")
                    for ec in range(2):
                        nc.tensor.matmul(
                            ops[:],
                            yt_sb[:, ec, tg : tg + 128],
                            wo_sb[:, ec, f0 : f0 + E // 2],
                            start=(ec == 0),
                            stop=(ec == 1),
                        )
                    if tail and fc == 0:
                        # spread the kernel-tail PSUM drain over ACT + DVE
                        nc.scalar.activation(ob[:, f0 : f0 + E // 2], ops[:], Act.Copy)
                    else:
                        nc.vector.tensor_copy(ob[:, f0 : f0 + E // 2], ops[:])
                    nc.sync.dma_start(
                        out_ap[tg : tg + 128, f0 : f0 + E // 2],
                        ob[:, f0 : f0 + E // 2],
                    )
                return [lambda: emit_fc(0), lambda: emit_fc(1)]

            def d_units(ic):
                units = []
                for g in range(IC // 128):
                    units += d_unit(ic, g)
                return units

            # ---- fused pipeline ----
            # dummy matmuls fill the PE while the first DMAs land (and keep
            # the HAM clock gate warm before the real work arrives)
            warm_mv = constp.tile([1, IC], bf16, tag="warm_mv")
            nc.vector.memset(warm_mv[:], 0.0)
            warm_ps = ps_sc.tile([128, 2, IC], f32, tag="st", name="warm_ps")
            for w in range(10):
                nc.tensor.matmul(
                    warm_ps[:, w % 2, :], ones1[:], warm_mv[:], start=True, stop=True
                )

            for u in b_units(0):
                u()

            for ic in range(N_IC):
                t0 = ic * IC
                if ic + 1 < N_IC:
                    dma_x(ic + 1)
                # filler units: next chunk's projections; output projections
                # deferred two chunks so the late (ACT-bound) attention
                # stretches still have PE work to absorb
                pending = []
                if ic + 1 < N_IC:
                    pending += b_units(ic + 1)
                if ic == N_IC - 1:
                    for dic in range(max(0, ic - 2), ic):
                        pending += d_units(dic)
                elif ic >= 2:
                    pending += d_units(ic - 2)
                njc = (ic + 1) * (IC // JC)
                slots = 2 * (njc + (4 if njc > 4 else 2)) + 4
                n_pend = len(pending)
                slot_i = 0
                emitted = 0

                for pair in range(2):
                    h0, h1 = 2 * pair, 2 * pair + 1
                    yp0 = ps_y.tile([DK + 1, IC], f32, tag="yp")
                    yp1 = ps_y.tile([DK + 1, IC], f32, tag="yp")
                    yps = [yp0, yp1]
                    pts = {}
                    # diagonal tiles (key chunk jc, o = jc - 4*ic >= 0): query
                    # columns < o*JC are fully masked -> computed only on the
                    # [o*JC:IC] column range; the triangular 0/1 mask applies
                    # to the first JC columns of that range.
                    LAG = 4 if njc > 4 else (2 if njc > 2 else 1)
                    c0s = {}
                    for jc in range(njc + LAG):
                        if jc < njc:
                            o = jc - (IC // JC) * ic
                            c0 = max(o, 0) * JC
                            c0s[jc] = c0
                            st2 = ps_sc.tile([128, 2, IC], f32, tag="st")
                            for i, r0 in enumerate((0, DK)):
                                nc.tensor.matmul(
                                    st2[:, i, c0:IC],
                                    kt_sb[r0 : r0 + DK, pair, jc * JC : (jc + 1) * JC],
                                    qt_sb[r0 : r0 + DK, pair, t0 + c0 : t0 + IC],
                                    start=True,
                                    stop=True,
                                )
                            pt = ptp.tile([128, 2, IC], bf16, tag="pt")
                            nc.scalar.activation(
                                pt[:, :, c0:IC], st2[:, :, c0:IC], Act.Exp, scale=0.125
                            )
                            if o >= 0:
                                nc.gpsimd.tensor_mul(
                                    pt[:, :, c0 : c0 + JC],
                                    pt[:, :, c0 : c0 + JC],
                                    masks_sb[:, 0:1, :].broadcast_to([128, 2, JC]),
                                )
                            pts[jc] = pt
                        slot_i += 1
                        while pending and emitted < (n_pend * slot_i) // slots:
                            pending.pop(0)()
                            emitted += 1
                        if jc >= LAG:  # prob @ [V | 1] for an earlier key tile
                            jp = jc - LAG
                            pt_prev = pts.pop(jp)
                            pc0 = c0s[jp]
                            for i, h in enumerate((h0, h1)):
                                nc.tensor.matmul(
                                    yps[i][:, pc0:IC],
                                    v1_sb[:, jp, h * 66 : h * 66 + 65],
                                    pt_prev[:, i, pc0:IC],
                                    start=(jp == 0),
                                    stop=(jp == njc - 1),
                                )
                    # normalize: rows 0:64 scaled by 1/denominator (row 64)
                    for i, h in enumerate((h0, h1)):
                        rcr = work.tile([1, IC], bf16, tag="rcr")
                        with nc.allow_low_precision(
                            reason="bf16 1/denom; 0.4% rel on softmax scale is fine"
                        ):
                            nc.vector.reciprocal(rcr[:], yps[i][DK : DK + 1, :])
                        bcs = work.tile([DK, IC], bf16, tag="bcs")
                        if ic == N_IC - 1 and pair == 1:
                            # kernel tail: PE + ACT are idle here and the
                            # recip->broadcast->mul chain gates phase D;
                            # Pool's partition_broadcast is the slowest link
                            bc = ps_w.tile([DK, IC], f32, tag="psb")
                            nc.tensor.matmul(
                                bc[:], ones64[:], rcr[:], start=True, stop=True
                            )
                            nc.scalar.activation(bcs[:], bc[:], Act.Copy)
                        else:
                            nc.gpsimd.partition_broadcast(bcs[:], rcr[:])
                        slot_i += 1
                        while pending and emitted < (n_pend * slot_i) // slots:
                            pending.pop(0)()
                            emitted += 1
                        r0 = (h % 2) * DK
                        nc.vector.tensor_mul(
                            yt_sb[r0 : r0 + DK, pair, t0 : t0 + IC],
                            yps[i][0:DK, :],
                            bcs[:],
                        )
                for u in pending:
                    u()

            for u in d_units(N_IC - 1):
                u()

            if debug_taps:
                for dst, src in ((dbg_qt, qt_sb), (dbg_kt, kt_sb), (dbg_yt, yt_sb)):
                    t = work.tile([128, 2, T], f32, tag="dbgf")
                    nc.vector.tensor_copy(t[:], src[:])
                    nc.sync.dma_start(dst[:], t[:])
                t = work.tile([128, N_TC, 4 * 66], f32, tag="dbgv")
                nc.vector.tensor_copy(t[:], v1_sb[:])
                nc.sync.dma_start(dbg_v1[:], t[:])

    nc.compile()
    return nc


def _masks():
    import ml_dtypes

    j = np.arange(JC)[:, None]
    i = np.arange(JC)[None, :]
    return np.where(j <= i, 1.0, 0.0).astype(ml_dtypes.bfloat16)


def _get_program():
    if "nc" not in _CACHE:
        _CACHE["nc"] = _build_program()
    return _CACHE["nc"]


def _prepare_in_maps(inputs):
    import ml_dtypes

    bf16 = ml_dtypes.bfloat16
    x = np.asarray(inputs["x"], dtype=np.float32).reshape(B, T, E)
    xT = [np.ascontiguousarray(x[b].T).astype(bf16) for b in range(B)]
    Wq = np.asarray(inputs["Wq"], dtype=np.float32)
    Wk = np.asarray(inputs["Wk"], dtype=np.float32)
    Wv = np.asarray(inputs["Wv"], dtype=np.float32)
    Wo = np.asarray(inputs["Wo"], dtype=np.float32)
    bq = np.asarray(inputs["bq"], dtype=np.float32)
    bk = np.asarray(inputs["bk"], dtype=np.float32)
    bv = np.asarray(inputs["bv"], dtype=np.float32)

    masks = _masks()
    in_maps = []
    for c in range(N_CORES):
        b, g = c // 4, c % 4
        sl = slice(g * F, (g + 1) * F)
        in_maps.append(
            {
                "x": xT[b],
                "wqT": np.ascontiguousarray(Wq[sl].T).astype(bf16),
                "wkT": np.ascontiguousarray(Wk[sl].T).astype(bf16),
                "wvT": np.ascontiguousarray(Wv[sl].T).astype(bf16),
                "woT": np.ascontiguousarray(Wo[:, sl].T).astype(bf16),
                "bq": np.ascontiguousarray(bq[sl]),
                "bk": np.ascontiguousarray(bk[sl]),
                "bv": np.ascontiguousarray(bv[sl]).astype(bf16),
                "masks": masks,
            }
        )
    return in_maps


def kernel(x, Wq, bq, Wk, bk, Wv, bv, Wo, bo):
    from concourse.bass_utils import run_bass_kernel_spmd

    nc = _get_program()
    bo = np.asarray(bo, dtype=np.float32)
    in_maps = _prepare_in_maps(
        {"x": x, "Wq": Wq, "bq": bq, "Wk": Wk, "bk": bk,
         "Wv": Wv, "bv": bv, "Wo": Wo, "bo": bo}
    )

    res = run_bass_kernel_spmd(nc, in_maps, core_ids=list(range(N_CORES)))
    out = np.zeros((B, T, E), dtype=np.float64)
    for c in range(N_CORES):
        out[c // 4] += np.asarray(res.results[c]["partial"], dtype=np.float64)
    out += bo[None, None, :]
    return out.astype(np.float32)

